# revision 28
# baseline (speedup 1.0000x reference)
"""Trainium2 Bass kernel for nn_DynamLinear: per-codebook linear -> chunked
outer product -> mean over codebooks -> RMS norm.

Math notes:
  ref: y = einsum('td,hdo->tho', x, W); split o=64 into a=y[..., :32], b=y[..., 32:]
       op[t,h,i,j] = a[t,h,i]*b[t,h,j];  out = mean_h(op)*sqrt(16); rms_norm(out)
  Since rms_norm is scale invariant, out = S / sqrt(mean(S^2) + 16e-12) where
       S[t,i,j] = sum_h a[t,h,i]*b[t,h,j]  (the per-token 16x32^T @ 16x32 matmul)

Per-core plan (tokens sharded 1024/core):
  warmup: dummy bf16 matmuls keep the PE p-state ramp warm while DMAs load.
  stage1: y^T = Wp^T @ x^T on TensorE (bf16), columns ordered so that a
          SBUF->SBUF DMA "shuffle" lands y into z[32r+h, sel, i, t256]
          (r = token/256, sel = a/b, t256 = token%256).  Shuffle DMAs are
          spread across SP/DVE/Pool queues (each DMA call holds its issuing
          engine for the whole modeled transfer time).
  stage2: per token one self-loading matmul lhsT=A_t[16h x 32i],
          rhs=B_t[16h x 32j] on a 32x32 PE tile (row group r, col group
          c = token%4) -> PSUM S_t[32i x 32j].
  rms:    ACT square (bf16 out), DVE reduce, indicator-matmul reduces over i
          (and broadcasts the per-token sums to all 128 partitions),
          sqrt+reciprocal, DVE multiply into a bf16 staging buffer, one
          batched store per quarter (bf16 halves the store traffic).
"""

import os
import sys
import functools
from contextlib import ExitStack

import numpy as np
import ml_dtypes

if "/opt/trn_rl_repo" not in sys.path:
    sys.path.insert(0, "/opt/trn_rl_repo")

import concourse.bass as bass
import concourse.bacc as bacc
import concourse.tile as tile
from concourse import mybir
from concourse.bass_utils import run_bass_kernel_spmd

N_CORES = 8
T_CORE = 1024          # tokens per core
D = 1024               # feat dim
H = 16                 # codebooks
EPS = 16e-12           # 16 * 1e-12 (scale-folded reference eps)

F32 = mybir.dt.float32
F32R = mybir.dt.float32r
BF16 = mybir.dt.bfloat16


def declare_io(nc):
    xt = nc.dram_tensor("xt", [128, 8, 1024], BF16, kind="ExternalInput").ap()
    wp = nc.dram_tensor("wp", [128, 8, 8, 128], BF16, kind="ExternalInput").ap()
    ind = nc.dram_tensor("ind", [128, 128], F32R, kind="ExternalInput").ap()
    out = nc.dram_tensor("out", [128, 16, 512], BF16, kind="ExternalOutput").ap()
    return xt, wp, ind, out


def _kernel_body(tc, ctx, xt, wp, ind, out):
    nc = tc.nc

    singles = ctx.enter_context(tc.tile_pool(name="singles", bufs=1))
    psum1 = ctx.enter_context(tc.tile_pool(name="psum1", bufs=3, space="PSUM"))
    psum2 = ctx.enter_context(tc.tile_pool(name="psum2", bufs=4, space="PSUM"))
    psum3 = ctx.enter_context(tc.tile_pool(name="psum3", bufs=1, space="PSUM"))
    sqpool = ctx.enter_context(tc.tile_pool(name="sqpool", bufs=2))
    sbfpool = ctx.enter_context(tc.tile_pool(name="sbfpool", bufs=3))
    obpool = ctx.enter_context(tc.tile_pool(name="obpool", bufs=2))
    smalls = ctx.enter_context(tc.tile_pool(name="smalls", bufs=4))

    # ---- warmup: keep PE busy through the p-state ramp while DMAs land ----
    wdum = singles.tile([128, 64], BF16)
    nc.vector.memset(wdum[:], 0.25)
    psw = psum1.tile([128, 256], F32, name="ps1")
    for i in range(56):
        nc.tensor.matmul(psw[0:64, 64 * (i % 4):64 * (i % 4) + 64],
                         lhsT=wdum[:], rhs=wdum[:],
                         start=True, stop=True, skip_group_check=True)

    # ---- resident inputs, loads spread over SP/ACT/Pool queues -----------
    wp_sb = singles.tile([128, 8, 8, 128], BF16)  # [dp, CT, dt, c7]
    xt_sb = singles.tile([128, 8, 1024], BF16)    # [dp, dt, t]: x^T
    ind_sb = singles.tile([128, 128], F32R)       # block-diag ones (4x 32x32)
    nc.sync.dma_start(out=wp_sb[:, 0:2], in_=wp[:, 0:2])
    nc.scalar.dma_start(out=xt_sb[:, :, 0:256], in_=xt[:, :, 0:256])
    nc.gpsimd.dma_start(out=wp_sb[:, 6:8], in_=wp[:, 6:8])
    nc.sync.dma_start(out=wp_sb[:, 4:6], in_=wp[:, 4:6])
    nc.scalar.dma_start(out=wp_sb[:, 2:4], in_=wp[:, 2:4])
    nc.gpsimd.dma_start(out=xt_sb[:, :, 512:768], in_=xt[:, :, 512:768])
    nc.scalar.dma_start(out=xt_sb[:, :, 256:512], in_=xt[:, :, 256:512])
    nc.sync.dma_start(out=xt_sb[:, :, 768:1024], in_=xt[:, :, 768:1024])
    nc.gpsimd.dma_start(out=ind_sb[:], in_=ind[:])
    eps_sb = singles.tile([128, 1], F32)
    nc.vector.memset(eps_sb[:], EPS)

    # ---- pipelined over token quarters tq (= row group r) ----------------
    y_sb = singles.tile([128, 8, 1024], BF16)    # [p, CT, t]
    z4 = singles.tile([128, 2, 4, 8, 256], BF16)
    out3 = out                                    # [p, ch, 512]
    ob_cell = [None]

    def _stage1(tq):
        t0q = tq * 256
        for CT in (0, 1, 6, 7, 2, 3, 4, 5):
            ps = psum1.tile([128, 256], F32, name="ps1")
            for d in range(8):
                nc.tensor.matmul(
                    ps[:],
                    lhsT=wp_sb[:, CT, d, :],
                    rhs=xt_sb[:, d, t0q:t0q + 256],
                    start=(d == 0),
                    stop=(d == 7),
                )
            if CT % 2 == 0:
                nc.scalar.activation(
                    y_sb[:, CT, t0q:t0q + 256], ps[:],
                    mybir.ActivationFunctionType.Copy,
                )
            else:
                with nc.allow_low_precision(reason="bf16 y copy"):
                    nc.vector.tensor_scalar_mul(
                        y_sb[:, CT, t0q:t0q + 256], ps[:], 1.0)
        # shuffle this quarter into z row group r = tq; spread engines.
        r = tq
        engs = (nc.sync, nc.gpsimd, nc.sync, nc.gpsimd,
                nc.sync, nc.gpsimd, nc.sync, nc.gpsimd)
        for m in range(8):
            engs[m].dma_start(
                out=z4[32 * r:32 * r + 16, :, :, m, :],
                in_=y_sb[16 * m:16 * m + 16, :, t0q:t0q + 256],
            )

    # ---- stage2 + rms, software-pipelined per 64-token chunk -------------
    ps2_of = {}
    part_of = {}
    ps3_of = {}
    sq_of = {}
    s_of = {}
    rstd_of = {}

    def _s2mm(ch, lo=0, hi=16):
        tq, half = ch // 4, ch % 4
        r, t0 = tq, 64 * (ch % 4)
        if lo == 0:
            ps2_of[ch] = psum2.tile([128, 16, 32], F32, name="ps2")
        ps2 = ps2_of[ch]
        for tw in range(4 * lo, 4 * hi):
            c, t32 = tw % 4, tw // 4
            t256 = t0 + tw
            nc.tensor.matmul(
                ps2[32 * c:32 * c + 32, t32, :],
                lhsT=z4[32 * r:32 * r + 16, 0, :, :, t256],
                rhs=z4[32 * r:32 * r + 16, 1, :, :, t256],
                start=True, stop=True,
                tile_position=(32 * r, 32 * c),
            )

    def _rms_a(ch, lo=0, hi=16):
        # square + per-group partial reduce (ACT -> DVE)
        ps2 = ps2_of[ch]
        sq = sqpool.tile([128, 16, 32], BF16, name="sq") if lo == 0 else \
            sq_of[ch]
        sq_of[ch] = sq
        nc.scalar.square(sq[:, lo:hi], ps2[:, lo:hi])
        part = smalls.tile([128, 16], F32R, name="part") if lo == 0 else \
            part_of[ch]
        part_of[ch] = part
        with nc.allow_low_precision(reason="f32r sum of 32 sq for rms"):
            nc.vector.tensor_reduce(part[:, lo:hi], sq[:, lo:hi],
                                    axis=mybir.AxisListType.X,
                                    op=mybir.AluOpType.add)

    def _rms_ind(ch, lo=0, hi=16):
        # indicator matmul: sum over i within 32-blocks + broadcast (PE)
        ps3 = psum3.tile([128, 16], F32, name="ps3") if lo == 0 else \
            ps3_of[ch]
        ps3_of[ch] = ps3
        nc.tensor.matmul(ps3[:, lo:hi], lhsT=ind_sb[:],
                         rhs=part_of[ch][:, lo:hi],
                         start=True, stop=True)

    def _rms_b(ch, lo=0, hi=16):
        tq, half = ch // 4, ch % 4
        if half == 0 and lo == 0:
            ob_cell[0] = obpool.tile([128, 4, 16, 32], BF16,
                                     name="obuf")
        obuf = ob_cell[0]
        ps3 = ps3_of[ch]
        s_sb = smalls.tile([128, 16], F32, name="s_sb") if lo == 0 else \
            s_of[ch]
        s_of[ch] = s_sb
        nc.scalar.activation(s_sb[:, lo:hi], ps3[:, lo:hi],
                             mybir.ActivationFunctionType.Sqrt,
                             bias=eps_sb[:], scale=1.0 / 1024.0)
        rstd = smalls.tile([128, 16], F32, name="rstd") if lo == 0 else \
            rstd_of[ch]
        rstd_of[ch] = rstd
        nc.vector.reciprocal(rstd[:, lo:hi], s_sb[:, lo:hi])
        with nc.allow_low_precision(reason="bf16 normalized output"):
            nc.vector.tensor_mul(
                obuf[:, half, lo:hi], ps2_of[ch][:, lo:hi],
                rstd[:, lo:hi].unsqueeze(2).broadcast_to(
                    [128, hi - lo, 32]))
        if hi == 16:
            if tq == 3 and half in (1, 2):
                q0, a0, n = (12, 0, 2) if half == 1 else (14, 2, 1)
                nc.gpsimd.dma_start(
                    out=out3[:, q0:q0 + n],
                    in_=obuf[:, a0:a0 + n].rearrange(
                        "p a b c -> p a (b c)"),
                )
            elif tq < 3 and half == 3:
                nc.gpsimd.dma_start(
                    out=out3[:, 4 * tq:4 * tq + 4],
                    in_=obuf[:].rearrange("p a b c -> p a (b c)"),
                )

    # issue plan: chunk pipeline runs one quarter behind stage1; the
    # ind-matmul for chunk ch is issued after chunk ch+1's stage2 matmuls so
    # the in-order PE queue never stalls waiting on the DVE reduce.
    done = []

    def _chunk_step(ch):
        if ch == 15:
            _rms_ind(14)
            _rms_b(14)
            for lo in (0, 8):
                _s2mm(ch, lo, lo + 8)
                _rms_a(ch, lo, lo + 8)
                _rms_ind(ch, lo, lo + 8)
                _rms_b(ch, lo, lo + 8)
                nc.gpsimd.dma_start(
                    out=out3[:, 15, 256 * (lo // 8):256 * (lo // 8) + 256],
                    in_=ob_cell[0][:, 3, lo:lo + 8].rearrange(
                        "p b c -> p (b c)"),
                )
            return
        _s2mm(ch)
        _rms_a(ch)
        if ch > 0:
            _rms_ind(ch - 1)
            _rms_b(ch - 1)
        done.append(ch)

    for tq in range(4):
        _stage1(tq)
    for ch in range(16):
        _chunk_step(ch)


@functools.lru_cache(maxsize=1)
def _build_program():
    nc = bacc.Bacc("TRN2", target_bir_lowering=False, debug=False)
    xt, wp, ind, out = declare_io(nc)
    with tile.TileContext(nc) as tc:
        with ExitStack() as ctx:
            _kernel_body(tc, ctx, xt, wp, ind, out)
    nc.compile()
    return nc


def _host_prep(x, weight):
    xf = np.ascontiguousarray(x.reshape(-1, D))          # [8192, 1024]
    # Wp column order: col = 512*sel + 128*ctp + 16*m + h ; i = 8*ctp + m
    w = weight.transpose(1, 0, 2).reshape(D, H, 2, 4, 8)  # [d, h, sel, ctp, m]
    wp = w.transpose(0, 2, 3, 4, 1).reshape(D, 1024)      # [d, col]
    wp_sb = np.ascontiguousarray(
        wp.reshape(8, 128, 8, 128).transpose(1, 2, 0, 3)).astype(
            ml_dtypes.bfloat16)
    ind = np.kron(np.eye(4, dtype=np.float32),
                  np.ones((32, 32), dtype=np.float32))
    xt_shards = []
    for c in range(N_CORES):
        xtc = xf[c * T_CORE:(c + 1) * T_CORE].T            # [d, t]
        xt_sb = np.ascontiguousarray(
            xtc.reshape(8, 128, 1024).transpose(1, 0, 2)).astype(
                ml_dtypes.bfloat16)
        xt_shards.append(xt_sb)
    return xt_shards, wp_sb, ind


def kernel(x, weight, **_unused):
    x = np.asarray(x, dtype=np.float32)
    weight = np.asarray(weight, dtype=np.float32)
    xt_shards, wp_sb, ind = _host_prep(x, weight)
    nc = _build_program()
    in_maps = [{"xt": xt_shards[c], "wp": wp_sb, "ind": ind}
               for c in range(N_CORES)]
    res = run_bass_kernel_spmd(nc, in_maps, list(range(N_CORES)))
    outs = []
    for c in range(N_CORES):
        d = np.asarray(res.results[c]["out"]).astype(np.float32)
        d = d.reshape(4, 32, 16, 16, 32)
        # [cg, i, ch, t32, j] -> token t = 128*ch + 4*t32 + cg, row = i*32+j
        outs.append(d.transpose(2, 3, 0, 1, 4).reshape(T_CORE, 1024))
    full = np.concatenate(outs, axis=0)                   # [8192, 1024]
    return full.reshape(x.shape[0], x.shape[1], 1024).astype(np.float32)


if __name__ == "__main__":
    rng = np.random.default_rng(0)
    x = rng.standard_normal((4, 2048, D), dtype=np.float32)
    w = (rng.standard_normal((H, D, 64), dtype=np.float32)
         * np.sqrt(2.0 / (D + 64))).astype(np.float32)
    o = kernel(x, w)
    print(o.shape, o.dtype)


# revision 29
# speedup vs baseline: 1.0281x; 1.0281x over previous
"""Trainium2 Bass kernel for nn_DynamLinear: per-codebook linear -> chunked
outer product -> mean over codebooks -> RMS norm.

Math notes:
  ref: y = einsum('td,hdo->tho', x, W); split o=64 into a=y[..., :32], b=y[..., 32:]
       op[t,h,i,j] = a[t,h,i]*b[t,h,j];  out = mean_h(op)*sqrt(16); rms_norm(out)
  Since rms_norm is scale invariant, out = S / sqrt(mean(S^2) + 16e-12) where
       S[t,i,j] = sum_h a[t,h,i]*b[t,h,j]  (the per-token 16x32^T @ 16x32 matmul)

Per-core plan (tokens sharded 1024/core):
  warmup: dummy bf16 matmuls keep the PE p-state ramp warm while DMAs load.
  stage1: y^T = Wp^T @ x^T on TensorE (bf16), columns ordered so that a
          SBUF->SBUF DMA "shuffle" lands y into z[32r+h, sel, i, t256]
          (r = token/256, sel = a/b, t256 = token%256).  Shuffle DMAs are
          spread across SP/DVE/Pool queues (each DMA call holds its issuing
          engine for the whole modeled transfer time).
  stage2: per token one self-loading matmul lhsT=A_t[16h x 32i],
          rhs=B_t[16h x 32j] on a 32x32 PE tile (row group r, col group
          c = token%4) -> PSUM S_t[32i x 32j].
  rms:    ACT square (bf16 out), DVE reduce, indicator-matmul reduces over i
          (and broadcasts the per-token sums to all 128 partitions),
          sqrt+reciprocal, DVE multiply into a bf16 staging buffer, one
          batched store per quarter (bf16 halves the store traffic).
"""

import os
import sys
import functools
from contextlib import ExitStack

import numpy as np
import ml_dtypes

if "/opt/trn_rl_repo" not in sys.path:
    sys.path.insert(0, "/opt/trn_rl_repo")

import concourse.bass as bass
import concourse.bacc as bacc
import concourse.tile as tile
from concourse import mybir
from concourse.bass_utils import run_bass_kernel_spmd

N_CORES = 8
T_CORE = 1024          # tokens per core
D = 1024               # feat dim
H = 16                 # codebooks
EPS = 16e-12           # 16 * 1e-12 (scale-folded reference eps)

F32 = mybir.dt.float32
F32R = mybir.dt.float32r
BF16 = mybir.dt.bfloat16


def declare_io(nc):
    xt = nc.dram_tensor("xt", [128, 8, 1024], BF16, kind="ExternalInput").ap()
    wp = nc.dram_tensor("wp", [128, 8, 8, 128], BF16, kind="ExternalInput").ap()
    ind = nc.dram_tensor("ind", [128, 128], F32R, kind="ExternalInput").ap()
    out = nc.dram_tensor("out", [128, 16, 512], BF16, kind="ExternalOutput").ap()
    return xt, wp, ind, out


def _kernel_body(tc, ctx, xt, wp, ind, out):
    nc = tc.nc

    singles = ctx.enter_context(tc.tile_pool(name="singles", bufs=1))
    psum1 = ctx.enter_context(tc.tile_pool(name="psum1", bufs=3, space="PSUM"))
    psum2 = ctx.enter_context(tc.tile_pool(name="psum2", bufs=4, space="PSUM"))
    psum3 = ctx.enter_context(tc.tile_pool(name="psum3", bufs=1, space="PSUM"))
    sqpool = ctx.enter_context(tc.tile_pool(name="sqpool", bufs=2))
    sbfpool = ctx.enter_context(tc.tile_pool(name="sbfpool", bufs=3))
    obpool = ctx.enter_context(tc.tile_pool(name="obpool", bufs=2))
    smalls = ctx.enter_context(tc.tile_pool(name="smalls", bufs=4))

    # ---- warmup: keep PE busy through the p-state ramp while DMAs land ----
    wdum = singles.tile([128, 64], BF16)
    nc.vector.memset(wdum[:], 0.25)
    psw = psum1.tile([128, 256], F32, name="ps1")
    for i in range(56):
        nc.tensor.matmul(psw[0:64, 64 * (i % 4):64 * (i % 4) + 64],
                         lhsT=wdum[:], rhs=wdum[:],
                         start=True, stop=True, skip_group_check=True)

    # ---- resident inputs, loads spread over SP/ACT/Pool queues -----------
    wp_sb = singles.tile([128, 8, 8, 128], BF16)  # [dp, CT, dt, c7]
    xt_sb = singles.tile([128, 8, 1024], BF16)    # [dp, dt, t]: x^T
    ind_sb = singles.tile([128, 128], F32R)       # block-diag ones (4x 32x32)
    nc.sync.dma_start(out=wp_sb[:, 0:2], in_=wp[:, 0:2])
    nc.scalar.dma_start(out=xt_sb[:, :, 0:256], in_=xt[:, :, 0:256])
    nc.gpsimd.dma_start(out=wp_sb[:, 6:8], in_=wp[:, 6:8])
    nc.sync.dma_start(out=wp_sb[:, 4:6], in_=wp[:, 4:6])
    nc.scalar.dma_start(out=wp_sb[:, 2:4], in_=wp[:, 2:4])
    nc.gpsimd.dma_start(out=xt_sb[:, :, 512:768], in_=xt[:, :, 512:768])
    nc.scalar.dma_start(out=xt_sb[:, :, 256:512], in_=xt[:, :, 256:512])
    nc.sync.dma_start(out=xt_sb[:, :, 768:1024], in_=xt[:, :, 768:1024])
    nc.gpsimd.dma_start(out=ind_sb[:], in_=ind[:])
    eps_sb = singles.tile([128, 1], F32)
    nc.vector.memset(eps_sb[:], EPS)

    # ---- pipelined over token quarters tq (= row group r) ----------------
    y_sb = singles.tile([128, 8, 1024], BF16)    # [p, CT, t]
    z4 = singles.tile([128, 2, 4, 8, 256], BF16)
    out3 = out                                    # [p, ch, 512]
    ob_cell = [None]

    def _stage1(tq):
        t0q = tq * 256
        for CT in (0, 1, 6, 7, 2, 3, 4, 5):
            ps = psum1.tile([128, 256], F32, name="ps1")
            for d in range(8):
                nc.tensor.matmul(
                    ps[:],
                    lhsT=wp_sb[:, CT, d, :],
                    rhs=xt_sb[:, d, t0q:t0q + 256],
                    start=(d == 0),
                    stop=(d == 7),
                )
            if CT % 2 == 0:
                nc.scalar.activation(
                    y_sb[:, CT, t0q:t0q + 256], ps[:],
                    mybir.ActivationFunctionType.Copy,
                )
            else:
                with nc.allow_low_precision(reason="bf16 y copy"):
                    nc.vector.tensor_scalar_mul(
                        y_sb[:, CT, t0q:t0q + 256], ps[:], 1.0)
        # shuffle this quarter into z row group r = tq; spread engines.
        r = tq
        engs = (nc.sync, nc.gpsimd, nc.sync, nc.gpsimd,
                nc.sync, nc.gpsimd, nc.sync, nc.gpsimd)
        for m in range(8):
            engs[m].dma_start(
                out=z4[32 * r:32 * r + 16, :, :, m, :],
                in_=y_sb[16 * m:16 * m + 16, :, t0q:t0q + 256],
            )

    # ---- stage2 + rms, software-pipelined per 64-token chunk -------------
    ps2_of = {}
    part_of = {}
    ps3_of = {}
    sq_of = {}
    s_of = {}
    rstd_of = {}

    def _s2mm(ch, lo=0, hi=16):
        tq, half = ch // 4, ch % 4
        r, t0 = tq, 64 * (ch % 4)
        if lo == 0:
            ps2_of[ch] = psum2.tile([128, 16, 32], F32, name="ps2")
        ps2 = ps2_of[ch]
        for tw in range(4 * lo, 4 * hi):
            c, t32 = tw % 4, tw // 4
            t256 = t0 + tw
            nc.tensor.matmul(
                ps2[32 * c:32 * c + 32, t32, :],
                lhsT=z4[32 * r:32 * r + 16, 0, :, :, t256],
                rhs=z4[32 * r:32 * r + 16, 1, :, :, t256],
                start=True, stop=True,
                tile_position=(32 * r, 32 * c),
            )

    def _rms_a(ch, lo=0, hi=16):
        # square + per-group partial reduce (ACT -> DVE)
        ps2 = ps2_of[ch]
        sq = sqpool.tile([128, 16, 32], BF16, name="sq") if lo == 0 else \
            sq_of[ch]
        sq_of[ch] = sq
        nc.scalar.square(sq[:, lo:hi], ps2[:, lo:hi])
        part = smalls.tile([128, 16], F32R, name="part") if lo == 0 else \
            part_of[ch]
        part_of[ch] = part
        with nc.allow_low_precision(reason="f32r sum of 32 sq for rms"):
            nc.vector.tensor_reduce(part[:, lo:hi], sq[:, lo:hi],
                                    axis=mybir.AxisListType.X,
                                    op=mybir.AluOpType.add)

    def _rms_ind(ch, lo=0, hi=16):
        # indicator matmul: sum over i within 32-blocks + broadcast (PE)
        ps3 = psum3.tile([128, 16], F32, name="ps3") if lo == 0 else \
            ps3_of[ch]
        ps3_of[ch] = ps3
        nc.tensor.matmul(ps3[:, lo:hi], lhsT=ind_sb[:],
                         rhs=part_of[ch][:, lo:hi],
                         start=True, stop=True)

    def _rms_b(ch, lo=0, hi=16):
        tq, half = ch // 4, ch % 4
        if half == 0 and lo == 0:
            ob_cell[0] = obpool.tile([128, 4, 16, 32], BF16,
                                     name="obuf")
        obuf = ob_cell[0]
        ps3 = ps3_of[ch]
        s_sb = smalls.tile([128, 16], F32, name="s_sb") if lo == 0 else \
            s_of[ch]
        s_of[ch] = s_sb
        nc.scalar.activation(s_sb[:, lo:hi], ps3[:, lo:hi],
                             mybir.ActivationFunctionType.Sqrt,
                             bias=eps_sb[:], scale=1.0 / 1024.0)
        rstd = smalls.tile([128, 16], F32, name="rstd") if lo == 0 else \
            rstd_of[ch]
        rstd_of[ch] = rstd
        nc.vector.reciprocal(rstd[:, lo:hi], s_sb[:, lo:hi])
        with nc.allow_low_precision(reason="bf16 normalized output"):
            nc.vector.tensor_mul(
                obuf[:, half, lo:hi], ps2_of[ch][:, lo:hi],
                rstd[:, lo:hi].unsqueeze(2).broadcast_to(
                    [128, hi - lo, 32]))
        if hi == 16:
            if tq == 3 and half in (1, 2):
                q0, a0, n = (12, 0, 2) if half == 1 else (14, 2, 1)
                nc.gpsimd.dma_start(
                    out=out3[:, q0:q0 + n],
                    in_=obuf[:, a0:a0 + n].rearrange(
                        "p a b c -> p a (b c)"),
                )
            elif tq < 3 and half == 3:
                nc.gpsimd.dma_start(
                    out=out3[:, 4 * tq:4 * tq + 4],
                    in_=obuf[:].rearrange("p a b c -> p a (b c)"),
                )

    # issue plan: chunk pipeline runs one quarter behind stage1; the
    # ind-matmul for chunk ch is issued after chunk ch+1's stage2 matmuls so
    # the in-order PE queue never stalls waiting on the DVE reduce.
    done = []

    def _chunk_step(ch):
        if ch == 15:
            _s2mm(ch, 0, 8)
            _rms_a(ch, 0, 8)
            _rms_ind(14)
            _rms_b(14)
            _s2mm(ch, 8, 16)
            _rms_a(ch, 8, 16)
            for lo in (0, 8):
                _rms_ind(ch, lo, lo + 8)
                _rms_b(ch, lo, lo + 8)
                nc.gpsimd.dma_start(
                    out=out3[:, 15, 256 * (lo // 8):256 * (lo // 8) + 256],
                    in_=ob_cell[0][:, 3, lo:lo + 8].rearrange(
                        "p b c -> p (b c)"),
                )
            return
        _s2mm(ch)
        _rms_a(ch)
        if ch > 0:
            _rms_ind(ch - 1)
            _rms_b(ch - 1)
        done.append(ch)

    for tq in range(4):
        _stage1(tq)
    for ch in range(16):
        _chunk_step(ch)


@functools.lru_cache(maxsize=1)
def _build_program():
    nc = bacc.Bacc("TRN2", target_bir_lowering=False, debug=False)
    xt, wp, ind, out = declare_io(nc)
    with tile.TileContext(nc) as tc:
        with ExitStack() as ctx:
            _kernel_body(tc, ctx, xt, wp, ind, out)
    nc.compile()
    return nc


def _host_prep(x, weight):
    xf = np.ascontiguousarray(x.reshape(-1, D))          # [8192, 1024]
    # Wp column order: col = 512*sel + 128*ctp + 16*m + h ; i = 8*ctp + m
    w = weight.transpose(1, 0, 2).reshape(D, H, 2, 4, 8)  # [d, h, sel, ctp, m]
    wp = w.transpose(0, 2, 3, 4, 1).reshape(D, 1024)      # [d, col]
    wp_sb = np.ascontiguousarray(
        wp.reshape(8, 128, 8, 128).transpose(1, 2, 0, 3)).astype(
            ml_dtypes.bfloat16)
    ind = np.kron(np.eye(4, dtype=np.float32),
                  np.ones((32, 32), dtype=np.float32))
    xt_shards = []
    for c in range(N_CORES):
        xtc = xf[c * T_CORE:(c + 1) * T_CORE].T            # [d, t]
        xt_sb = np.ascontiguousarray(
            xtc.reshape(8, 128, 1024).transpose(1, 0, 2)).astype(
                ml_dtypes.bfloat16)
        xt_shards.append(xt_sb)
    return xt_shards, wp_sb, ind


def kernel(x, weight, **_unused):
    x = np.asarray(x, dtype=np.float32)
    weight = np.asarray(weight, dtype=np.float32)
    xt_shards, wp_sb, ind = _host_prep(x, weight)
    nc = _build_program()
    in_maps = [{"xt": xt_shards[c], "wp": wp_sb, "ind": ind}
               for c in range(N_CORES)]
    res = run_bass_kernel_spmd(nc, in_maps, list(range(N_CORES)))
    outs = []
    for c in range(N_CORES):
        d = np.asarray(res.results[c]["out"]).astype(np.float32)
        d = d.reshape(4, 32, 16, 16, 32)
        # [cg, i, ch, t32, j] -> token t = 128*ch + 4*t32 + cg, row = i*32+j
        outs.append(d.transpose(2, 3, 0, 1, 4).reshape(T_CORE, 1024))
    full = np.concatenate(outs, axis=0)                   # [8192, 1024]
    return full.reshape(x.shape[0], x.shape[1], 1024).astype(np.float32)


if __name__ == "__main__":
    rng = np.random.default_rng(0)
    x = rng.standard_normal((4, 2048, D), dtype=np.float32)
    w = (rng.standard_normal((H, D, 64), dtype=np.float32)
         * np.sqrt(2.0 / (D + 64))).astype(np.float32)
    o = kernel(x, w)
    print(o.shape, o.dtype)
